# revision 29
# baseline (speedup 1.0000x reference)
"""Expert-parallel MoE MLP kernel for Trainium2 (8 NeuronCores, 1 expert/core).

Problem: inputs [1, 8, 16384, 512], per-expert 2-layer GELU MLP
  h   = gelu(x @ W1[e] + b1[e])      # [16384, 2048]
  out = h @ W2[e] + b2[e]            # [16384, 512]

Dataflow (all matmul operands bf16; the PE runs ONLY the 128 N=512
matmuls per 512-token block, at the 216ns/matmul hardware floor):
  1. Host casts x/W1/W2 to bf16 (rel err ~3e-3, gate is 2e-2; bf16 also
     enables FWL fast weight loads -> LDWEIGHTS fully hidden).
  2. One batched xbar-transpose DMA per block lands xT [d_p, k, t] in
     SBUF (k-outer d layout), fully off the PE.
  3. L1: psum[f,t] = sum_k matmul(lhsT=W1[dk, f], rhs=xT[dk, t])
  4. ScalarE Gelu(+b1 per-partition bias) psum -> hT sbuf [f, t] bf16
  5. L2: psum[t,d] = sum_k matmul(lhsT=hT[fk, t], rhs=W2[fk, d])
     -> natural token-major layout, no output transpose
  6. DVE adds b2 (broadcast) psum -> f32 sbuf, one batched store per
     block (per-j for the last block to shrink the tail).

All DMAs go on the SP queue with minimal instruction count: the tile
scheduler serializes every DMA against the xbar transposes into one
chain, where a same-queue link costs ~50ns but a cross-queue link ~3us.
Block 0 is the exception: its x block is loaded in natural layout and
transposed on the PE.  That keeps the startup free of xbar transposes
(so the startup DMAs pipeline at dispatch rate instead of chaining on
completions), and the 16 PE transposes double as the HAM warmup burst
(1.2 -> 2.4 GHz), ending exactly when the real matmul stream becomes
ready -- no warmup-vs-DMA timing race.
"""

import os
import numpy as np

E, C, D, F = 8, 16384, 512, 2048
P = 128
TBLK = 512  # tokens per block
KD = D // P   # 4  k-tiles (d) for layer 1
KF = F // P   # 16 k-tiles (f) for layer 2
JT = TBLK // P  # 4 token sub-tiles per block

_CACHE = {}


def _build(T, act="Gelu_apprx_tanh"):
    import concourse.mybir as mybir
    import concourse.tile as tile
    from concourse import bacc
    from concourse.masks import make_identity

    f32 = mybir.dt.float32
    bf16 = mybir.dt.bfloat16
    gelu_fn = getattr(mybir.ActivationFunctionType, act)

    nc = bacc.Bacc("TRN2", target_bir_lowering=False, debug=False)

    x_d = nc.dram_tensor("x", [T, D], bf16, kind="ExternalInput").ap()
    w1_d = nc.dram_tensor("w1", [D, F], bf16, kind="ExternalInput").ap()
    b1_d = nc.dram_tensor("b1", [F], f32, kind="ExternalInput").ap()
    w2_d = nc.dram_tensor("w2", [F, D], bf16, kind="ExternalInput").ap()
    b2_d = nc.dram_tensor("b2", [D], f32, kind="ExternalInput").ap()
    o_d = nc.dram_tensor("out", [T, D], f32, kind="ExternalOutput").ap()

    NB = T // TBLK

    with tile.TileContext(nc) as tc:
        with (
            tc.tile_pool(name="consts", bufs=1) as consts,
            tc.tile_pool(name="xt", bufs=4) as xt_pool,
            tc.tile_pool(name="ht", bufs=1) as ht_pool,
            tc.tile_pool(name="ot", bufs=2) as ot_pool,
            tc.tile_pool(name="ph", bufs=4, space="PSUM") as ph_pool,
            tc.tile_pool(name="po", bufs=2, space="PSUM") as po_pool,
            tc.tile_pool(name="pxt", bufs=2, space="PSUM") as pxt_pool,
        ):
            # --- setup.  All DMAs serialize into one chain against the
            # xbar-transpose DMAs (tile scheduler), and each cross-queue
            # chain link costs ~3us of semaphore latency vs ~50ns same-queue.
            # So: every DMA on the SP queue, minimal instruction count (one
            # DMA per tensor, one xbar + one store per block), ordered so
            # the consumer-side need times are met: w1, b1, xt0, w2, xt1,
            # b2, xt2, xt3. ---
            #
            # The batched xbar writes x's d-column c into (k=c//P, p=c%P)
            # of the [P, KD, TBLK] tile (k-outer), so partition p of k-slice
            # k holds d = k*P + p -- the natural k-tile layout.
            # Block 0's x arrives as a NATURAL-layout load and is transposed
            # on the PE.  Two wins: the startup has no xbar transpose, so
            # all startup DMAs pipeline at dispatch rate instead of chaining
            # on completions; and the PE transposes double as the HAM warmup
            # burst, ending exactly when the real stream can start (no
            # warmup-vs-DMA timing race, which cost ~3-7us on unlucky runs).
            ident = consts.tile([P, P], bf16)
            make_identity(nc, ident[:])
            xn0 = consts.tile([P, JT, D], bf16)
            nc.sync.dma_start(
                xn0[:], x_d[0:TBLK, :].rearrange("(j p) d -> p j d", p=P)
            )
            b1_sb = consts.tile([P, KF], f32)
            nc.sync.dma_start(b1_sb[:], b1_d.rearrange("(k p) -> p k", p=P))

            # w1 arrives in 512-col f-chunks: the first L1 f-group only
            # waits on chunk 0 (512KB).
            w1_sb = consts.tile([P, KD, F], bf16)
            w1_r = w1_d.rearrange("(k p) f -> p k f", p=P)
            FC = 512
            for fc in range(F // FC):
                nc.sync.dma_start(
                    w1_sb[:, :, fc * FC : (fc + 1) * FC],
                    w1_r[:, :, fc * FC : (fc + 1) * FC],
                )

            w2_sb = consts.tile([P, KF, D], bf16)
            w2_r = w2_d.rearrange("(k p) d -> p k d", p=P)
            nc.sync.dma_start(w2_sb[:], w2_r)
            b2_bc = consts.tile([P, D], f32)
            nc.sync.dma_start(b2_bc[:], b2_d.unsqueeze(0).partition_broadcast(P))

            def load_xt(blk):
                """One xbar-transposed DMA: x[t0:t0+TBLK, :] -> [d_p, k, t]."""
                t0 = blk * TBLK
                xt = xt_pool.tile([P, KD, TBLK], bf16, name="xt", tag="xt")
                nc.sync.dma_start(
                    xt[:], x_d[t0 : t0 + TBLK, :], transpose=True
                )
                return xt

            # PE-transpose block 0 into the same [d_p, k, t] layout the
            # xbar produces for blocks 1+.
            xt0 = xt_pool.tile([P, KD, TBLK], bf16, name="xt", tag="xt")
            for k in range(KD):
                pxt = pxt_pool.tile([P, TBLK], bf16)
                for j in range(JT):
                    nc.tensor.transpose(
                        pxt[:, j * P : (j + 1) * P],
                        xn0[:, j, k * P : (k + 1) * P],
                        ident[:],
                    )
                nc.vector.tensor_copy(xt0[:, k, :], pxt[:])
            xts = {0: xt0}
            xts[1] = load_xt(1)
            xts[2] = load_xt(2)
            xts[3] = load_xt(3)

            def layer1(xt_cur):
                hts = []
                for f in range(KF):
                    ph = ph_pool.tile([P, TBLK], f32)
                    for k in range(KD):
                        nc.tensor.matmul(
                            ph[:],
                            w1_sb[:, k, f * P : (f + 1) * P],
                            xt_cur[:, k, :],
                            start=(k == 0),
                            stop=(k == KD - 1),
                        )
                    ht_f = ht_pool.tile(
                        [P, TBLK], bf16, name=f"ht{f}", tag=f"ht{f}"
                    )
                    nc.scalar.activation(
                        ht_f[:], ph[:], gelu_fn, bias=b1_sb[:, f : f + 1]
                    )
                    hts.append(ht_f)
                return hts

            def layer2(blk, hts):
                t0 = blk * TBLK
                ot = ot_pool.tile([P, JT, D], f32, name="ot", tag="ot")
                for j in range(JT):
                    po = po_pool.tile([P, D], f32)
                    for k in range(KF):
                        nc.tensor.matmul(
                            po[:],
                            hts[k][:, j * P : (j + 1) * P],
                            w2_sb[:, k, :],
                            start=(k == 0),
                            stop=(k == KF - 1),
                        )
                    if blk == NB - 1:
                        # last block: store per-j (halving the final j) so
                        # only ~128KB sits in the post-matmul tail
                        H = D // 2
                        splits = (
                            [(0, H), (H, D)] if j == JT - 1 else [(0, D)]
                        )
                        for d0, d1 in splits:
                            nc.vector.tensor_add(
                                ot[:, j, d0:d1], po[:, d0:d1], b2_bc[:, d0:d1]
                            )
                            nc.sync.dma_start(
                                o_d[t0 + j * P : t0 + (j + 1) * P, d0:d1],
                                ot[:, j, d0:d1],
                            )
                    else:
                        nc.vector.tensor_add(ot[:, j, :], po[:], b2_bc[:])
                if blk < NB - 1:
                    nc.sync.dma_start(
                        o_d[t0 : t0 + TBLK, :].rearrange("(j p) d -> p j d", p=P),
                        ot[:],
                    )

            for blk in range(NB):
                if blk + 4 < NB:
                    xts[blk + 4] = load_xt(blk + 4)
                hts = layer1(xts.pop(blk))
                layer2(blk, hts)

    nc.compile()
    return nc


def _get_nc(T):
    if T not in _CACHE:
        _CACHE[T] = _build(T)
    return _CACHE[T]


def kernel(inputs, W1, b1, W2, b2):
    import ml_dtypes
    from concourse.bass_utils import run_bass_kernel_spmd

    bf16 = ml_dtypes.bfloat16
    inputs = np.asarray(inputs, dtype=np.float32).astype(bf16)
    W1 = np.asarray(W1, dtype=np.float32).astype(bf16)
    b1 = np.ascontiguousarray(np.asarray(b1, dtype=np.float32))
    W2 = np.asarray(W2, dtype=np.float32).astype(bf16)
    b2 = np.ascontiguousarray(np.asarray(b2, dtype=np.float32))

    nc = _get_nc(C)
    in_maps = [
        {
            "x": np.ascontiguousarray(inputs[0, e]),
            "w1": np.ascontiguousarray(W1[e]),
            "b1": b1[e],
            "w2": np.ascontiguousarray(W2[e]),
            "b2": b2[e],
        }
        for e in range(E)
    ]
    trace = os.environ.get("KERNEL_TRACE", "0") == "1"
    res = run_bass_kernel_spmd(
        nc, in_maps, core_ids=list(range(E)), trace=trace
    )
    if trace:
        kernel.last_exec_time_ns = res.exec_time_ns
    out = np.stack([res.results[e]["out"] for e in range(E)], axis=0)[None]
    return out


# revision 31
# speedup vs baseline: 1.0006x; 1.0006x over previous
"""Expert-parallel MoE MLP kernel for Trainium2 (8 NeuronCores, 1 expert/core).

Problem: inputs [1, 8, 16384, 512], per-expert 2-layer GELU MLP
  h   = gelu(x @ W1[e] + b1[e])      # [16384, 2048]
  out = h @ W2[e] + b2[e]            # [16384, 512]

Dataflow (all matmul operands bf16; the PE runs ONLY the 128 N=512
matmuls per 512-token block, at the 216ns/matmul hardware floor):
  1. Host casts x/W1/W2 to bf16 (rel err ~3e-3, gate is 2e-2; bf16 also
     enables FWL fast weight loads -> LDWEIGHTS fully hidden).
  2. One batched xbar-transpose DMA per block lands xT [d_p, k, t] in
     SBUF (k-outer d layout), fully off the PE.
  3. L1: psum[f,t] = sum_k matmul(lhsT=W1[dk, f], rhs=xT[dk, t])
  4. ScalarE Gelu(+b1 per-partition bias) psum -> hT sbuf [f, t] bf16
  5. L2: psum[t,d] = sum_k matmul(lhsT=hT[fk, t], rhs=W2[fk, d])
     -> natural token-major layout, no output transpose
  6. DVE adds b2 (broadcast) psum -> f32 sbuf, one batched store per
     block (per-j for the last block to shrink the tail).

All DMAs go on the SP queue with minimal instruction count: the tile
scheduler serializes every DMA against the xbar transposes into one
chain, where a same-queue link costs ~50ns but a cross-queue link ~3us.
Block 0 is the exception: its x block is loaded in natural layout and
transposed on the PE.  That keeps the startup free of xbar transposes
(so the startup DMAs pipeline at dispatch rate instead of chaining on
completions), and the 16 PE transposes double as the HAM warmup burst
(1.2 -> 2.4 GHz), ending exactly when the real matmul stream becomes
ready -- no warmup-vs-DMA timing race.
"""

import os
import numpy as np

E, C, D, F = 8, 16384, 512, 2048
P = 128
TBLK = 512  # tokens per block
KD = D // P   # 4  k-tiles (d) for layer 1
KF = F // P   # 16 k-tiles (f) for layer 2
JT = TBLK // P  # 4 token sub-tiles per block

_CACHE = {}


def _build(T, act="Gelu_apprx_tanh"):
    import concourse.mybir as mybir
    import concourse.tile as tile
    from concourse import bacc
    from concourse.masks import make_identity

    f32 = mybir.dt.float32
    bf16 = mybir.dt.bfloat16
    gelu_fn = getattr(mybir.ActivationFunctionType, act)

    nc = bacc.Bacc("TRN2", target_bir_lowering=False, debug=False)

    x_d = nc.dram_tensor("x", [T, D], bf16, kind="ExternalInput").ap()
    w1_d = nc.dram_tensor("w1", [D, F], bf16, kind="ExternalInput").ap()
    b1_d = nc.dram_tensor("b1", [F], f32, kind="ExternalInput").ap()
    w2_d = nc.dram_tensor("w2", [F, D], bf16, kind="ExternalInput").ap()
    b2_d = nc.dram_tensor("b2", [D], f32, kind="ExternalInput").ap()
    o_d = nc.dram_tensor("out", [T, D], f32, kind="ExternalOutput").ap()

    NB = T // TBLK

    with tile.TileContext(nc) as tc:
        with (
            tc.tile_pool(name="consts", bufs=1) as consts,
            tc.tile_pool(name="xt", bufs=4) as xt_pool,
            tc.tile_pool(name="ht", bufs=1) as ht_pool,
            tc.tile_pool(name="ot", bufs=2) as ot_pool,
            tc.tile_pool(name="ph", bufs=4, space="PSUM") as ph_pool,
            tc.tile_pool(name="po", bufs=2, space="PSUM") as po_pool,
            tc.tile_pool(name="pxt", bufs=2, space="PSUM") as pxt_pool,
        ):
            # --- setup.  All DMAs serialize into one chain against the
            # xbar-transpose DMAs (tile scheduler), and each cross-queue
            # chain link costs ~3us of semaphore latency vs ~50ns same-queue.
            # So: every DMA on the SP queue, minimal instruction count (one
            # DMA per tensor, one xbar + one store per block), ordered so
            # the consumer-side need times are met: w1, b1, xt0, w2, xt1,
            # b2, xt2, xt3. ---
            #
            # The batched xbar writes x's d-column c into (k=c//P, p=c%P)
            # of the [P, KD, TBLK] tile (k-outer), so partition p of k-slice
            # k holds d = k*P + p -- the natural k-tile layout.
            # Block 0's x arrives as a NATURAL-layout load and is transposed
            # on the PE.  Two wins: the startup has no xbar transpose, so
            # all startup DMAs pipeline at dispatch rate instead of chaining
            # on completions; and the PE transposes double as the HAM warmup
            # burst, ending exactly when the real stream can start (no
            # warmup-vs-DMA timing race, which cost ~3-7us on unlucky runs).
            ident = consts.tile([P, P], bf16)
            make_identity(nc, ident[:])
            xn0 = consts.tile([P, JT, D], bf16)
            nc.sync.dma_start(
                xn0[:], x_d[0:TBLK, :].rearrange("(j p) d -> p j d", p=P)
            )
            # w1 arrives in f-chunks, smallest first: L1 f-group 0 only
            # waits on a 256KB chunk whose completion (~13.3us) lines up
            # with the end of the block-0 transposes, not after it.
            w1_sb = consts.tile([P, KD, F], bf16)
            w1_r = w1_d.rearrange("(k p) f -> p k f", p=P)
            nc.sync.dma_start(w1_sb[:, :, 0:256], w1_r[:, :, 0:256])
            b1_sb = consts.tile([P, KF], f32)
            nc.sync.dma_start(b1_sb[:], b1_d.rearrange("(k p) -> p k", p=P))
            nc.sync.dma_start(w1_sb[:, :, 256:512], w1_r[:, :, 256:512])
            FC = 512
            for fc in range(1, F // FC):
                nc.sync.dma_start(
                    w1_sb[:, :, fc * FC : (fc + 1) * FC],
                    w1_r[:, :, fc * FC : (fc + 1) * FC],
                )

            w2_sb = consts.tile([P, KF, D], bf16)
            w2_r = w2_d.rearrange("(k p) d -> p k d", p=P)
            nc.sync.dma_start(w2_sb[:], w2_r)
            b2_bc = consts.tile([P, D], f32)
            nc.sync.dma_start(b2_bc[:], b2_d.unsqueeze(0).partition_broadcast(P))

            def load_xt(blk):
                """One xbar-transposed DMA: x[t0:t0+TBLK, :] -> [d_p, k, t]."""
                t0 = blk * TBLK
                xt = xt_pool.tile([P, KD, TBLK], bf16, name="xt", tag="xt")
                nc.sync.dma_start(
                    xt[:], x_d[t0 : t0 + TBLK, :], transpose=True
                )
                return xt

            # PE-transpose block 0 into the same [d_p, k, t] layout the
            # xbar produces for blocks 1+.
            xt0 = xt_pool.tile([P, KD, TBLK], bf16, name="xt", tag="xt")
            for k in range(KD):
                pxt = pxt_pool.tile([P, TBLK], bf16)
                for j in range(JT):
                    nc.tensor.transpose(
                        pxt[:, j * P : (j + 1) * P],
                        xn0[:, j, k * P : (k + 1) * P],
                        ident[:],
                    )
                nc.vector.tensor_copy(xt0[:, k, :], pxt[:])
            xts = {0: xt0}
            xts[1] = load_xt(1)
            xts[2] = load_xt(2)
            xts[3] = load_xt(3)

            def layer1(xt_cur):
                hts = []
                for f in range(KF):
                    ph = ph_pool.tile([P, TBLK], f32)
                    for k in range(KD):
                        nc.tensor.matmul(
                            ph[:],
                            w1_sb[:, k, f * P : (f + 1) * P],
                            xt_cur[:, k, :],
                            start=(k == 0),
                            stop=(k == KD - 1),
                        )
                    ht_f = ht_pool.tile(
                        [P, TBLK], bf16, name=f"ht{f}", tag=f"ht{f}"
                    )
                    nc.scalar.activation(
                        ht_f[:], ph[:], gelu_fn, bias=b1_sb[:, f : f + 1]
                    )
                    hts.append(ht_f)
                return hts

            def layer2(blk, hts):
                t0 = blk * TBLK
                ot = ot_pool.tile([P, JT, D], f32, name="ot", tag="ot")
                for j in range(JT):
                    po = po_pool.tile([P, D], f32)
                    if blk == NB - 1 and j == JT - 1:
                        # very last j-tile: two independent N=256
                        # accumulation chains, so the first half's add +
                        # store overlap the second half's matmuls and only
                        # ~128KB sits in the post-matmul tail
                        H = D // 2
                        for d0, d1 in ((0, H), (H, D)):
                            for k in range(KF):
                                nc.tensor.matmul(
                                    po[:, d0:d1],
                                    hts[k][:, j * P : (j + 1) * P],
                                    w2_sb[:, k, d0:d1],
                                    start=(k == 0),
                                    stop=(k == KF - 1),
                                )
                            nc.vector.tensor_add(
                                ot[:, j, d0:d1], po[:, d0:d1], b2_bc[:, d0:d1]
                            )
                            nc.sync.dma_start(
                                o_d[t0 + j * P : t0 + (j + 1) * P, d0:d1],
                                ot[:, j, d0:d1],
                            )
                        continue
                    for k in range(KF):
                        nc.tensor.matmul(
                            po[:],
                            hts[k][:, j * P : (j + 1) * P],
                            w2_sb[:, k, :],
                            start=(k == 0),
                            stop=(k == KF - 1),
                        )
                    if blk == NB - 1:
                        # last block: store per-j so stores drain during
                        # the remaining matmuls
                        nc.vector.tensor_add(ot[:, j, :], po[:], b2_bc[:])
                        nc.sync.dma_start(
                            o_d[t0 + j * P : t0 + (j + 1) * P, :],
                            ot[:, j, :],
                        )
                    else:
                        nc.vector.tensor_add(ot[:, j, :], po[:], b2_bc[:])
                if blk < NB - 1:
                    nc.sync.dma_start(
                        o_d[t0 : t0 + TBLK, :].rearrange("(j p) d -> p j d", p=P),
                        ot[:],
                    )

            for blk in range(NB):
                if blk + 4 < NB:
                    xts[blk + 4] = load_xt(blk + 4)
                hts = layer1(xts.pop(blk))
                layer2(blk, hts)

    nc.compile()
    return nc


def _get_nc(T):
    if T not in _CACHE:
        _CACHE[T] = _build(T)
    return _CACHE[T]


def kernel(inputs, W1, b1, W2, b2):
    import ml_dtypes
    from concourse.bass_utils import run_bass_kernel_spmd

    bf16 = ml_dtypes.bfloat16
    inputs = np.asarray(inputs, dtype=np.float32).astype(bf16)
    W1 = np.asarray(W1, dtype=np.float32).astype(bf16)
    b1 = np.ascontiguousarray(np.asarray(b1, dtype=np.float32))
    W2 = np.asarray(W2, dtype=np.float32).astype(bf16)
    b2 = np.ascontiguousarray(np.asarray(b2, dtype=np.float32))

    nc = _get_nc(C)
    in_maps = [
        {
            "x": np.ascontiguousarray(inputs[0, e]),
            "w1": np.ascontiguousarray(W1[e]),
            "b1": b1[e],
            "w2": np.ascontiguousarray(W2[e]),
            "b2": b2[e],
        }
        for e in range(E)
    ]
    trace = os.environ.get("KERNEL_TRACE", "0") == "1"
    res = run_bass_kernel_spmd(
        nc, in_maps, core_ids=list(range(E)), trace=trace
    )
    if trace:
        kernel.last_exec_time_ns = res.exec_time_ns
    out = np.stack([res.results[e]["out"] for e in range(E)], axis=0)[None]
    return out


# revision 33
# speedup vs baseline: 1.0041x; 1.0035x over previous
"""Expert-parallel MoE MLP kernel for Trainium2 (8 NeuronCores, 1 expert/core).

Problem: inputs [1, 8, 16384, 512], per-expert 2-layer GELU MLP
  h   = gelu(x @ W1[e] + b1[e])      # [16384, 2048]
  out = h @ W2[e] + b2[e]            # [16384, 512]

Dataflow (all matmul operands bf16; the PE runs ONLY the 128 N=512
matmuls per 512-token block, at the 216ns/matmul hardware floor):
  1. Host casts x/W1/W2 to bf16 (rel err ~3e-3, gate is 2e-2; bf16 also
     enables FWL fast weight loads -> LDWEIGHTS fully hidden).
  2. One batched xbar-transpose DMA per block lands xT [d_p, k, t] in
     SBUF (k-outer d layout), fully off the PE.
  3. L1: psum[f,t] = sum_k matmul(lhsT=W1[dk, f], rhs=xT[dk, t])
  4. ScalarE Gelu(+b1 per-partition bias) psum -> hT sbuf [f, t] bf16
  5. L2: psum[t,d] = sum_k matmul(lhsT=hT[fk, t], rhs=W2[fk, d])
     -> natural token-major layout, no output transpose
  6. DVE adds b2 (broadcast) psum -> f32 sbuf, one batched store per
     block (per-j for the last block to shrink the tail).

All DMAs go on the SP queue with minimal instruction count: the tile
scheduler serializes every DMA against the xbar transposes into one
chain, where a same-queue link costs ~50ns but a cross-queue link ~3us.
Block 0 is the exception: its x block is loaded in natural layout and
transposed on the PE.  That keeps the startup free of xbar transposes
(so the startup DMAs pipeline at dispatch rate instead of chaining on
completions), and the 16 PE transposes double as the HAM warmup burst
(1.2 -> 2.4 GHz), ending exactly when the real matmul stream becomes
ready -- no warmup-vs-DMA timing race.
"""

import os
import numpy as np

E, C, D, F = 8, 16384, 512, 2048
P = 128
TBLK = 512  # tokens per block
KD = D // P   # 4  k-tiles (d) for layer 1
KF = F // P   # 16 k-tiles (f) for layer 2
JT = TBLK // P  # 4 token sub-tiles per block

_CACHE = {}


def _build(T, act="Gelu_apprx_tanh"):
    import concourse.mybir as mybir
    import concourse.tile as tile
    from concourse import bacc
    from concourse.masks import make_identity

    f32 = mybir.dt.float32
    bf16 = mybir.dt.bfloat16
    gelu_fn = getattr(mybir.ActivationFunctionType, act)

    nc = bacc.Bacc("TRN2", target_bir_lowering=False, debug=False)

    x_d = nc.dram_tensor("x", [T, D], bf16, kind="ExternalInput").ap()
    w1_d = nc.dram_tensor("w1", [D, F], bf16, kind="ExternalInput").ap()
    b1_d = nc.dram_tensor("b1", [F], f32, kind="ExternalInput").ap()
    w2_d = nc.dram_tensor("w2", [F, D], bf16, kind="ExternalInput").ap()
    b2_d = nc.dram_tensor("b2", [D], f32, kind="ExternalInput").ap()
    o_d = nc.dram_tensor("out", [T, D], f32, kind="ExternalOutput").ap()

    NB = T // TBLK

    with tile.TileContext(nc) as tc:
        with (
            tc.tile_pool(name="consts", bufs=1) as consts,
            tc.tile_pool(name="xt", bufs=4) as xt_pool,
            tc.tile_pool(name="ht", bufs=1) as ht_pool,
            tc.tile_pool(name="ot", bufs=2) as ot_pool,
            tc.tile_pool(name="ph", bufs=4, space="PSUM") as ph_pool,
            tc.tile_pool(name="po", bufs=2, space="PSUM") as po_pool,
            tc.tile_pool(name="pxt", bufs=2, space="PSUM") as pxt_pool,
        ):
            # --- setup.  All DMAs serialize into one chain against the
            # xbar-transpose DMAs (tile scheduler), and each cross-queue
            # chain link costs ~3us of semaphore latency vs ~50ns same-queue.
            # So: every DMA on the SP queue, minimal instruction count (one
            # DMA per tensor, one xbar + one store per block), ordered so
            # the consumer-side need times are met: w1, b1, xt0, w2, xt1,
            # b2, xt2, xt3. ---
            #
            # The batched xbar writes x's d-column c into (k=c//P, p=c%P)
            # of the [P, KD, TBLK] tile (k-outer), so partition p of k-slice
            # k holds d = k*P + p -- the natural k-tile layout.
            # Block 0's x arrives as a NATURAL-layout load and is transposed
            # on the PE.  Two wins: the startup has no xbar transpose, so
            # all startup DMAs pipeline at dispatch rate instead of chaining
            # on completions; and the PE transposes double as the HAM warmup
            # burst, ending exactly when the real stream can start (no
            # warmup-vs-DMA timing race, which cost ~3-7us on unlucky runs).
            ident = consts.tile([P, P], bf16)
            make_identity(nc, ident[:])
            # block 0's x in two half-loads: the PE transposes the first
            # half while the second half's DMA is still in flight
            xn0 = consts.tile([P, JT, D], bf16)
            JH = JT // 2
            for h in range(2):
                nc.sync.dma_start(
                    xn0[:, h * JH : (h + 1) * JH, :],
                    x_d[h * JH * P : (h + 1) * JH * P, :].rearrange(
                        "(j p) d -> p j d", p=P
                    ),
                )
            # w1 arrives in f-chunks, smallest first: L1 f-group 0 only
            # waits on a 256KB chunk whose completion (~13.3us) lines up
            # with the end of the block-0 transposes, not after it.
            w1_sb = consts.tile([P, KD, F], bf16)
            w1_r = w1_d.rearrange("(k p) f -> p k f", p=P)
            nc.sync.dma_start(w1_sb[:, :, 0:256], w1_r[:, :, 0:256])
            b1_sb = consts.tile([P, KF], f32)
            nc.sync.dma_start(b1_sb[:], b1_d.rearrange("(k p) -> p k", p=P))
            nc.sync.dma_start(w1_sb[:, :, 256:512], w1_r[:, :, 256:512])
            FC = 512
            for fc in range(1, F // FC):
                nc.sync.dma_start(
                    w1_sb[:, :, fc * FC : (fc + 1) * FC],
                    w1_r[:, :, fc * FC : (fc + 1) * FC],
                )

            w2_sb = consts.tile([P, KF, D], bf16)
            w2_r = w2_d.rearrange("(k p) d -> p k d", p=P)
            nc.sync.dma_start(w2_sb[:], w2_r)
            b2_bc = consts.tile([P, D], f32)
            nc.sync.dma_start(b2_bc[:], b2_d.unsqueeze(0).partition_broadcast(P))

            def load_xt(blk):
                """One xbar-transposed DMA: x[t0:t0+TBLK, :] -> [d_p, k, t]."""
                t0 = blk * TBLK
                xt = xt_pool.tile([P, KD, TBLK], bf16, name="xt", tag="xt")
                nc.sync.dma_start(
                    xt[:], x_d[t0 : t0 + TBLK, :], transpose=True
                )
                return xt

            # PE-transpose block 0 into the same [d_p, k, t] layout the
            # xbar produces for blocks 1+, half-major so transposing the
            # first token half overlaps the second half's load.
            xt0 = xt_pool.tile([P, KD, TBLK], bf16, name="xt", tag="xt")
            HW_ = JH * P  # tokens per half
            for h in range(2):
                for k in range(KD):
                    pxt = pxt_pool.tile([P, HW_], bf16)
                    for jj in range(JH):
                        j = h * JH + jj
                        nc.tensor.transpose(
                            pxt[:, jj * P : (jj + 1) * P],
                            xn0[:, j, k * P : (k + 1) * P],
                            ident[:],
                        )
                    nc.vector.tensor_copy(
                        xt0[:, k, h * HW_ : (h + 1) * HW_], pxt[:]
                    )
            xts = {0: xt0}
            xts[1] = load_xt(1)
            xts[2] = load_xt(2)
            xts[3] = load_xt(3)

            def layer1(xt_cur):
                hts = []
                for f in range(KF):
                    ph = ph_pool.tile([P, TBLK], f32)
                    for k in range(KD):
                        nc.tensor.matmul(
                            ph[:],
                            w1_sb[:, k, f * P : (f + 1) * P],
                            xt_cur[:, k, :],
                            start=(k == 0),
                            stop=(k == KD - 1),
                        )
                    ht_f = ht_pool.tile(
                        [P, TBLK], bf16, name=f"ht{f}", tag=f"ht{f}"
                    )
                    nc.scalar.activation(
                        ht_f[:], ph[:], gelu_fn, bias=b1_sb[:, f : f + 1]
                    )
                    hts.append(ht_f)
                return hts

            def layer2(blk, hts):
                t0 = blk * TBLK
                ot = ot_pool.tile([P, JT, D], f32, name="ot", tag="ot")
                for j in range(JT):
                    po = po_pool.tile([P, D], f32)
                    if blk == NB - 1 and j == JT - 1:
                        # very last j-tile: two independent N=256
                        # accumulation chains, so the first half's add +
                        # store overlap the second half's matmuls and only
                        # ~128KB sits in the post-matmul tail
                        H = D // 2
                        for d0, d1 in ((0, H), (H, D)):
                            for k in range(KF):
                                nc.tensor.matmul(
                                    po[:, d0:d1],
                                    hts[k][:, j * P : (j + 1) * P],
                                    w2_sb[:, k, d0:d1],
                                    start=(k == 0),
                                    stop=(k == KF - 1),
                                )
                            nc.vector.tensor_add(
                                ot[:, j, d0:d1], po[:, d0:d1], b2_bc[:, d0:d1]
                            )
                            nc.sync.dma_start(
                                o_d[t0 + j * P : t0 + (j + 1) * P, d0:d1],
                                ot[:, j, d0:d1],
                            )
                        continue
                    for k in range(KF):
                        nc.tensor.matmul(
                            po[:],
                            hts[k][:, j * P : (j + 1) * P],
                            w2_sb[:, k, :],
                            start=(k == 0),
                            stop=(k == KF - 1),
                        )
                    if blk == NB - 1:
                        # last block: store per-j so stores drain during
                        # the remaining matmuls
                        nc.vector.tensor_add(ot[:, j, :], po[:], b2_bc[:])
                        nc.sync.dma_start(
                            o_d[t0 + j * P : t0 + (j + 1) * P, :],
                            ot[:, j, :],
                        )
                    else:
                        nc.vector.tensor_add(ot[:, j, :], po[:], b2_bc[:])
                if blk < NB - 1:
                    nc.sync.dma_start(
                        o_d[t0 : t0 + TBLK, :].rearrange("(j p) d -> p j d", p=P),
                        ot[:],
                    )

            for blk in range(NB):
                if blk + 4 < NB:
                    xts[blk + 4] = load_xt(blk + 4)
                hts = layer1(xts.pop(blk))
                layer2(blk, hts)

    nc.compile()
    return nc


def _get_nc(T):
    if T not in _CACHE:
        _CACHE[T] = _build(T)
    return _CACHE[T]


def kernel(inputs, W1, b1, W2, b2):
    import ml_dtypes
    from concourse.bass_utils import run_bass_kernel_spmd

    bf16 = ml_dtypes.bfloat16
    inputs = np.asarray(inputs, dtype=np.float32).astype(bf16)
    W1 = np.asarray(W1, dtype=np.float32).astype(bf16)
    b1 = np.ascontiguousarray(np.asarray(b1, dtype=np.float32))
    W2 = np.asarray(W2, dtype=np.float32).astype(bf16)
    b2 = np.ascontiguousarray(np.asarray(b2, dtype=np.float32))

    nc = _get_nc(C)
    in_maps = [
        {
            "x": np.ascontiguousarray(inputs[0, e]),
            "w1": np.ascontiguousarray(W1[e]),
            "b1": b1[e],
            "w2": np.ascontiguousarray(W2[e]),
            "b2": b2[e],
        }
        for e in range(E)
    ]
    trace = os.environ.get("KERNEL_TRACE", "0") == "1"
    res = run_bass_kernel_spmd(
        nc, in_maps, core_ids=list(range(E)), trace=trace
    )
    if trace:
        kernel.last_exec_time_ns = res.exec_time_ns
    out = np.stack([res.results[e]["out"] for e in range(E)], axis=0)[None]
    return out
